# revision 21
# baseline (speedup 1.0000x reference)
"""Trainium2 Bass kernel for nn_ExpertBlock (dense transformer block with
outer-product mixes). 8-core token-parallel SPMD: core c handles batch c//2,
token half c%2 (1024 q-tokens each); K/V computed for the full 2048-token
batch on each core. No collectives.

Layout: feature-major activations hT [D=128 partitions, tokens].
Key tricks:
  - LayerNorm stats via PE ones-matmul column sums; rsqrt = exp(-0.5*ln(v+eps))
    so everything stays in the natural_log_exp ACT table set with softmax Exp.
  - Attention scores computed transposed [k_pos, q] with K=16 row-tiled
    matmul pairs; softmax denominator comes free from a ones-row appended to
    V (col-tiled ctx matmul, 4 heads per PSUM tile); padding mask folded in
    as the per-partition bias of the Exp activation.
  - Op-mix t_i*t_j Linear via circulant diagonals: P_d[i,n] = t[i,n]*t[(i+d)%128,n]
    for d=0..64 (symmetry-folded host-side into the weight), formed by
    partition-shifted SBUF->SBUF DMA copies + one bf16 tensor_tensor per
    diagonal, contracted on PE with pairs already on partitions.
"""
import os
import sys

sys.path.insert(0, "/opt/trn_rl_repo")

import numpy as np
import ml_dtypes
from contextlib import ExitStack

import concourse.bass as bass
import concourse.mybir as mybir
import concourse.tile as tile
from concourse import bacc

BF16 = mybir.dt.bfloat16
F32 = mybir.dt.float32
AF = mybir.ActivationFunctionType
ALU = mybir.AluOpType

B, N, D, H, FF = 4, 2048, 128, 8, 512
HD = D // H  # 16
EPS = 1e-5
NCORES = 8
TOK = N // 2  # q tokens per core (1024)
NKC = N // 128  # 16 kpos chunks
NDIAG = 65  # circulant diagonals 0..64

bf = ml_dtypes.bfloat16

_CACHE = {}


# ---------------------------------------------------------------------------
# host-side weight prep
# ---------------------------------------------------------------------------
def _prep_weights(inp):
    w = {}
    Wqkv = np.asarray(inp["Wqkv"], np.float32)
    bqkv = np.asarray(inp["bqkv"], np.float32)
    Wq, Wk, Wv = Wqkv[0:D], Wqkv[D : 2 * D], Wqkv[2 * D : 3 * D]
    bq, bk, bv = bqkv[0:D], bqkv[D : 2 * D], bqkv[2 * D : 3 * D]
    sc = 1.0 / np.sqrt(np.float32(HD))
    w["wq"] = np.ascontiguousarray(Wq.T).astype(bf)
    w["wk"] = np.ascontiguousarray((Wk * sc).T).astype(bf)  # fold 1/sqrt(hd)
    w["wv"] = np.ascontiguousarray(Wv.T).astype(bf)
    w["bq"] = bq.reshape(D, 1).astype(np.float32)
    w["bk"] = (bk * sc).reshape(D, 1).astype(np.float32)
    w["bv"] = bv.reshape(D, 1).astype(np.float32)

    # out-proj in "spread" layout: head hg*4+hp, dim j at partition 32*hp+j
    Wo = np.asarray(inp["Wo"], np.float32)
    # denominator row sits at partition 32*hp (j=0 slot); head dims at +1..+16
    wo_sp = np.zeros((D, 2, D), np.float32)  # [partition, hg, dout]
    for hg in range(2):
        for hp in range(4):
            for j in range(HD):
                wo_sp[32 * hp + 1 + j, hg, :] = Wo[:, HD * (4 * hg + hp) + j]
    w["wo_sp"] = wo_sp.reshape(D, 2 * D).astype(bf)
    w["bo"] = np.asarray(inp["bo"], np.float32).reshape(D, 1)

    w["w1t"] = np.ascontiguousarray(np.asarray(inp["ffn_W1"], np.float32).T).astype(bf)
    w["b1"] = np.ascontiguousarray(
        np.asarray(inp["ffn_b1"], np.float32).reshape(4, 128).T
    )
    W2t = np.asarray(inp["ffn_W2"], np.float32).T.reshape(4, 128, D)  # [fc, f, dout]
    w["w2t"] = np.ascontiguousarray(np.transpose(W2t, (1, 0, 2)).reshape(128, 4 * D)).astype(bf)
    w["b2"] = np.asarray(inp["ffn_b2"], np.float32).reshape(D, 1)

    # opmix circulant fold: out[k,n] = sum_d sum_i Wd[d][k,i]*t[i,n]*t[(i+d)%128,n]
    idx = np.arange(D)
    for nm, wn, bn in (("op1", "wop1", "ob1"), ("op2", "wop2", "ob2")):
        G = np.asarray(inp[nm + "_W"], np.float32).reshape(D, D, D)  # [k,i,j]
        Wd = np.zeros((NDIAG, D, D), np.float32)  # [d, k, i]
        Wd[0] = G[:, idx, idx]
        for d in range(1, 64):
            j = (idx + d) % D
            Wd[d] = G[:, idx, j] + np.transpose(G, (0, 2, 1))[:, idx, j]
        j64 = (idx + 64) % D
        Wd[64] = G[:, idx, j64]
        # lhsT_d[i, k] = Wd[d][k, i]; store [i, d*128+k]
        lhsT = np.transpose(Wd, (2, 0, 1)).reshape(D, NDIAG * D)
        w[wn] = np.ascontiguousarray(lhsT).astype(bf)
        w[bn] = np.asarray(inp[nm + "_b"], np.float32).reshape(D, 1)

    g = np.stack(
        [
            np.asarray(inp["ln_a_g"], np.float32),
            np.asarray(inp["ln_op1_g"], np.float32),
            np.asarray(inp["ln_mlp_g"], np.float32),
            np.asarray(inp["ln_op2_g"], np.float32),
        ]
    )  # [4, 128]
    bta = np.stack(
        [
            np.asarray(inp["ln_a_b"], np.float32),
            np.asarray(inp["ln_op1_b"], np.float32),
            np.asarray(inp["ln_mlp_b"], np.float32),
            np.asarray(inp["ln_op2_b"], np.float32),
        ]
    )
    w["ln_g"] = np.ascontiguousarray(g.T)  # [128, 4]
    w["ln_b"] = np.ascontiguousarray(bta.T)
    w["ln_grow"] = np.ascontiguousarray(g.reshape(1, 4 * D))  # [1, 512]
    w["ln_nbrow"] = np.ascontiguousarray((-bta).reshape(1, 4 * D))

    w["c_inv128"] = np.full((D, 1), 1.0 / D, np.float32)
    w["c_onesrow"] = np.ones((1, 512), np.float32)
    w["c_eps"] = np.full((1, 1), EPS, np.float32)
    w["ident"] = np.eye(D, dtype=np.float32).astype(bf)
    w["c_ones"] = np.ones((D, 32), np.float32)
    return w


_INPUT_SPECS = [
    ("hT", (D, N), F32),
    ("hTq", (D, TOK), F32),
    ("maskb", (D, NKC), F32),
    ("wq", (D, D), BF16),
    ("wk", (D, D), BF16),
    ("wv", (D, D), BF16),
    ("bq", (D, 1), F32),
    ("bk", (D, 1), F32),
    ("bv", (D, 1), F32),
    ("wo_sp", (D, 2 * D), BF16),
    ("bo", (D, 1), F32),
    ("w1t", (D, FF), BF16),
    ("b1", (D, 4), F32),
    ("w2t", (D, 4 * D), BF16),
    ("b2", (D, 1), F32),
    ("wop1", (D, NDIAG * D), BF16),
    ("ob1", (D, 1), F32),
    ("wop2", (D, NDIAG * D), BF16),
    ("ob2", (D, 1), F32),
    ("ln_g", (D, 4), F32),
    ("ln_b", (D, 4), F32),
    ("ln_grow", (1, 4 * D), F32),
    ("ln_nbrow", (1, 4 * D), F32),
    ("c_inv128", (D, 1), F32),
    ("c_onesrow", (1, 512), F32),
    ("c_eps", (1, 1), F32),
    ("ident", (D, D), BF16),
    ("c_ones", (D, 32), F32),
]


def _per_core_inputs(inp, w):
    h = np.asarray(inp["h"], np.float32)
    mask = np.asarray(inp["key_padding_mask"])
    maps = []
    for c in range(NCORES):
        b, half = c // 2, c % 2
        m = dict(w)
        m["hT"] = np.ascontiguousarray(h[b].T)  # [128, 2048]
        m["hTq"] = np.ascontiguousarray(h[b].T[:, half * TOK : (half + 1) * TOK])
        mb = np.where(mask[b], np.float32(-1e9), np.float32(0.0))
        m["maskb"] = np.ascontiguousarray(mb.reshape(NKC, 128).T)  # [128, 16]
        maps.append(m)
    return maps


# ---------------------------------------------------------------------------
# device kernel
# ---------------------------------------------------------------------------
def build_kernel():
    nc = bacc.Bacc("TRN2", target_bir_lowering=False, debug=False, num_devices=NCORES)
    p = {}
    for nm, shape, dt in _INPUT_SPECS:
        p[nm] = nc.declare_dram_parameter(nm, list(shape), dt, isOutput=False)
    p["outT"] = nc.declare_dram_parameter("outT", [D, TOK], F32, isOutput=True)

    with ExitStack() as ctx:
        tc = ctx.enter_context(tile.TileContext(nc))
        const = ctx.enter_context(tc.tile_pool(name="const", bufs=1))
        hpool = ctx.enter_context(tc.tile_pool(name="hpool", bufs=1))
        work = ctx.enter_context(tc.tile_pool(name="work", bufs=2))
        expp = ctx.enter_context(tc.tile_pool(name="expp", bufs=6))
        shp = ctx.enter_context(tc.tile_pool(name="shp", bufs=4))
        pdp = ctx.enter_context(tc.tile_pool(name="pdp", bufs=4))
        ps_sc = ctx.enter_context(tc.tile_pool(name="ps_sc", bufs=1, space="PSUM"))
        ps_ctx = ctx.enter_context(tc.tile_pool(name="ps_ctx", bufs=1, space="PSUM"))
        ps_op = ctx.enter_context(tc.tile_pool(name="ps_op", bufs=2, space="PSUM"))
        ps_sm = ctx.enter_context(tc.tile_pool(name="ps_sm", bufs=2, space="PSUM"))

        # ---- load constants / inputs ------------------------------------
        ct = {}
        for nm, shape, dt in _INPUT_SPECS:
            if nm in ("hT", "hTq"):
                continue
            t = const.tile(list(shape), dt, tag=nm)
            nc.sync.dma_start(t[:, :], p[nm][:, :])
            ct[nm] = t

        hT = hpool.tile([D, N], F32, tag="hT")
        nc.sync.dma_start(hT[:, :], p["hT"][:, :])
        hTq = hpool.tile([D, TOK], F32, tag="hTq")
        nc.sync.dma_start(hTq[:, :], p["hTq"][:, :])

        # ---- LayerNorm chunk: dst[:, :512] (bf16) = LN(src[:, :512]) -----
        def ln_chunk(dst_ap, src_ap, li):
            sq = work.tile([D, 512], F32, tag="sq")
            nc.scalar.activation(sq[:, :], src_ap, AF.Square)
            st = ps_sm.tile([D, 512], F32, tag="sm")
            nc.tensor.matmul(st[0:1, :], ct["c_inv128"][:, :], src_ap)
            nc.tensor.matmul(
                st[32:33, :], ct["c_inv128"][:, :], sq[:, :], tile_position=(0, 32)
            )
            mu_sb = work.tile([2, 512], F32, tag="lnrow")
            nc.scalar.copy(mu_sb[0:1, :], st[0:1, :])
            musq = work.tile([2, 512], F32, tag="lnrow2")
            nc.vector.tensor_mul(musq[0:1, :], mu_sb[0:1, :], st[0:1, :])
            var = work.tile([2, 512], F32, tag="lnrow3")
            nc.vector.tensor_sub(var[0:1, :], st[32:33, :], musq[0:1, :])
            # r = rsqrt(var + eps) = exp(-0.5 * ln(var + eps))
            lv = work.tile([2, 512], F32, tag="lnrow4")
            nc.scalar.activation(lv[0:1, :], var[0:1, :], AF.Ln, bias=ct["c_eps"][:, :])
            r_sb = work.tile([2, 512], F32, tag="lnrow5")
            nc.scalar.activation(r_sb[0:1, :], lv[0:1, :], AF.Exp, scale=-0.5)
            c_sb = work.tile([2, 512], F32, tag="lnrow6")
            nc.vector.tensor_mul(c_sb[0:1, :], mu_sb[0:1, :], r_sb[0:1, :])
            # broadcasts: Rb = ones.T @ r ; Dg = g.T @ c + (-b).T @ ones
            Rb = ps_sm.tile([D, 512], F32, tag="sm")
            nc.tensor.matmul(Rb[:, :], ct["c_onesrow"][:, 0:128], r_sb[0:1, :])
            Dg = ps_sm.tile([D, 512], F32, tag="sm")
            nc.tensor.matmul(
                Dg[:, :],
                ct["ln_grow"][:, 128 * li : 128 * (li + 1)],
                c_sb[0:1, :],
                start=True,
                stop=False,
            )
            nc.tensor.matmul(
                Dg[:, :],
                ct["ln_nbrow"][:, 128 * li : 128 * (li + 1)],
                ct["c_onesrow"][:, :],
                start=False,
                stop=True,
            )
            x2 = work.tile([D, 512], F32, tag="x2")
            nc.vector.tensor_mul(x2[:, :], src_ap, Rb[:, :])
            # t = x2 * g - Dg
            nc.vector.scalar_tensor_tensor(
                dst_ap,
                x2[:, :],
                ct["ln_g"][:, li : li + 1],
                Dg[:, :],
                ALU.mult,
                ALU.subtract,
            )

        # ---- phase 1: LN_a ------------------------------------------------
        ta_full = hpool.tile([D, N], BF16, tag="ta_full")
        for c in range(4):
            ln_chunk(ta_full[:, 512 * c : 512 * (c + 1)], hT[:, 512 * c : 512 * (c + 1)], 0)
        ta_q = hpool.tile([D, TOK], BF16, tag="ta_q")
        for c in range(2):
            ln_chunk(ta_q[:, 512 * c : 512 * (c + 1)], hTq[:, 512 * c : 512 * (c + 1)], 0)

        # ---- phase 2: qkv projections ------------------------------------
        kT = hpool.tile([D, N], BF16, tag="kT")
        vT = hpool.tile([D, N], BF16, tag="vT")
        qT = hpool.tile([D, TOK], BF16, tag="qT")
        for c in range(4):
            sl = slice(512 * c, 512 * (c + 1))
            for wnm, bnm, dst in (("wk", "bk", kT), ("wv", "bv", vT)):
                pj = ps_sm.tile([D, 512], F32, tag="sm")
                nc.tensor.matmul(pj[:, :], ct[wnm][:, :], ta_full[:, sl])
                nc.scalar.activation(
                    dst[:, sl], pj[:, :], AF.Identity, bias=ct[bnm][:, :]
                )
        for c in range(2):
            sl = slice(512 * c, 512 * (c + 1))
            pj = ps_sm.tile([D, 512], F32, tag="sm")
            nc.tensor.matmul(pj[:, :], ct["wq"][:, :], ta_q[:, sl])
            nc.scalar.activation(qT[:, sl], pj[:, :], AF.Identity, bias=ct["bq"][:, :])

        # ---- phase 3: V transpose + V_aug [128, 16*256] -------------------
        vaug = hpool.tile([D, NKC * 256], BF16, tag="vaug")
        nc.gpsimd.memset(vaug[:, :], 0.0)
        for kc in range(NKC):
            tp = ps_ctx.tile([D, 128], BF16, tag="ctx")
            nc.tensor.transpose(
                tp[:, :], vT[:, 128 * kc : 128 * (kc + 1)], ct["ident"][:, :]
            )
            seg = vaug[:, 256 * kc : 256 * (kc + 1)].rearrange(
                "p (h j) -> p h j", j=32
            )
            nc.vector.tensor_copy(
                seg[:, :, 1:17],
                tp[:, 0:128].rearrange("p (h j) -> p h j", j=16),
            )
            nc.vector.memset(seg[:, :, 0:1], 1.0)

        # ---- phase 4: row-group stagings for scores ----------------------
        kT4 = [
            hpool.tile([D, N], BF16, tag=f"kT4_{s}", name=f"kT4_{s}") for s in range(2)
        ]
        qT4 = [
            hpool.tile([D, TOK], BF16, tag=f"qT4_{s}", name=f"qT4_{s}")
            for s in range(2)
        ]
        for s in range(2):
            for g in range(4):
                hh = 4 * s + g
                nc.sync.dma_start(
                    kT4[s][32 * g : 32 * g + 16, :], kT[16 * hh : 16 * hh + 16, :]
                )
                nc.sync.dma_start(
                    qT4[s][32 * g : 32 * g + 16, :], qT[16 * hh : 16 * hh + 16, :]
                )

        # ---- residual adds helper ----------------------------------------
        def resid(dst_ap, psum_ap, bias_ap, prev_ap):
            # dst = (psum + bias_pp) + prev
            nc.vector.scalar_tensor_tensor(
                dst_ap, psum_ap, bias_ap, prev_ap, ALU.add, ALU.add
            )

        # ---- phase 5: attention ------------------------------------------
        h1 = hpool.tile([D, TOK], F32, tag="h1")
        for qh in range(2):
            qsl = slice(512 * qh, 512 * (qh + 1))
            mha = ps_sm.tile([D, 512], F32, tag="sm", name=f"mha_{qh}")
            for hg in range(2):
                s = hg  # staging s holds heads 4s..4s+3
                # scores + exp + ctx interleaved per kpos chunk
                cx = ps_ctx.tile([D, 512], F32, tag="ctx")
                for kc in range(NKC):
                    ksl = slice(128 * kc, 128 * (kc + 1))
                    ets = []
                    for pi in range(2):
                        b0, b1 = (0, 32) if pi == 0 else (64, 96)
                        sc = ps_sc.tile([D, 1024], F32, tag="sc")
                        nc.tensor.matmul(
                            sc[:, 0:512],
                            kT4[s][b0 : b0 + 16, ksl],
                            qT4[s][b0 : b0 + 16, qsl],
                            tile_position=(b0, 0),
                        )
                        nc.tensor.matmul(
                            sc[:, 512:1024],
                            kT4[s][b1 : b1 + 16, ksl],
                            qT4[s][b1 : b1 + 16, qsl],
                            tile_position=(b1, 0),
                        )
                        et = expp.tile([D, 1024], BF16, tag="exp")
                        nc.scalar.activation(
                            et[:, :], sc[:, :], AF.Exp, bias=ct["maskb"][:, kc : kc + 1]
                        )
                        ets.append(et)
                    for hp in range(4):
                        hh = 4 * hg + hp
                        nc.tensor.matmul(
                            cx[32 * hp : 32 * hp + 32, :],
                            vaug[:, 256 * kc + 32 * hh : 256 * kc + 32 * hh + 32],
                            ets[hp // 2][:, 512 * (hp % 2) : 512 * (hp % 2) + 512],
                            start=(kc == 0),
                            stop=(kc == NKC - 1),
                            tile_position=(0, 32 * hp),
                            skip_group_check=True,
                        )
                # softmax normalize: recip of denom rows (partitions 32*hp),
                # then broadcast each row over its 32-block via K=1 matmuls
                rc = work.tile([D, 512], F32, tag="recip")
                for hp in range(4):
                    nc.vector.reciprocal(
                        rc[32 * hp : 32 * hp + 1, :], cx[32 * hp : 32 * hp + 1, :]
                    )
                rb = ps_sm.tile([D, 512], F32, tag="sm")
                for hp in range(4):
                    nc.tensor.matmul(
                        rb[32 * hp : 32 * hp + 32, :],
                        ct["c_ones"][32 * hp : 32 * hp + 1, :],
                        rc[32 * hp : 32 * hp + 1, :],
                        tile_position=(32 * hp, 32 * hp),
                        skip_group_check=True,
                    )
                rb_sb = work.tile([D, 512], F32, tag="recipb")
                nc.scalar.copy(rb_sb[:, :], rb[:, :])
                csp = work.tile([D, 512], BF16, tag="ctxsp")
                nc.vector.tensor_mul(csp[:, :], cx[:, :], rb_sb[:, :])
                # out-proj accumulate over hgroups
                nc.tensor.matmul(
                    mha[:, :],
                    ct["wo_sp"][:, 128 * hg : 128 * (hg + 1)],
                    csp[:, :],
                    start=(hg == 0),
                    stop=(hg == 1),
                )
            resid(h1[:, qsl], mha[:, :], ct["bo"][:, :], hTq[:, qsl])

        # ---- op-mix ------------------------------------------------------
        def opmix(h_in, wnm, bnm, li, h_out, tnm):
            t_op = hpool.tile([D, TOK], BF16, tag=tnm)
            for c in range(2):
                sl = slice(512 * c, 512 * (c + 1))
                ln_chunk(t_op[:, sl], h_in[:, sl], li)
            ops = [
                ps_op.tile([D, 512], F32, tag="op", name=f"op_{tnm}_{qc}")
                for qc in range(2)
            ]
            for d in range(NDIAG):
                if d == 0:
                    pd = pdp.tile([D, TOK], BF16, tag="pd")
                    nc.vector.tensor_mul(pd[:, :], t_op[:, :], t_op[:, :])
                else:
                    bd = shp.tile([D, TOK], BF16, tag="bd")
                    nc.sync.dma_start(bd[0 : D - d, :], t_op[d:D, :])
                    nc.sync.dma_start(bd[D - d : D, :], t_op[0:d, :])
                    pd = pdp.tile([D, TOK], BF16, tag="pd")
                    nc.vector.tensor_mul(pd[:, :], t_op[:, :], bd[:, :])
                for qc in range(2):
                    nc.tensor.matmul(
                        ops[qc][:, :],
                        ct[wnm][:, 128 * d : 128 * (d + 1)],
                        pd[:, 512 * qc : 512 * (qc + 1)],
                        start=(d == 0),
                        stop=(d == NDIAG - 1),
                    )
            for qc in range(2):
                sl = slice(512 * qc, 512 * (qc + 1))
                resid(h_out[:, sl], ops[qc][:, :], ct[bnm][:, :], h_in[:, sl])

        h2 = hpool.tile([D, TOK], F32, tag="h2")
        opmix(h1, "wop1", "ob1", 1, h2, "t1")

        # ---- FFN ---------------------------------------------------------
        h3 = hpool.tile([D, TOK], F32, tag="h3")
        tm = hpool.tile([D, TOK], BF16, tag="tm")
        for c in range(2):
            sl = slice(512 * c, 512 * (c + 1))
            ln_chunk(tm[:, sl], h2[:, sl], 2)
        for qc in range(2):
            sl = slice(512 * qc, 512 * (qc + 1))
            f2 = ps_op.tile([D, 512], F32, tag="op")
            for fc in range(4):
                f1 = ps_op.tile([D, 512], F32, tag="ff1", bufs=1)
                nc.tensor.matmul(
                    f1[:, :], ct["w1t"][:, 128 * fc : 128 * (fc + 1)], tm[:, sl]
                )
                gl = work.tile([D, 512], BF16, tag="gelu")
                gelu_f = AF.Identity if os.environ.get("SIM_GELU_ID") else AF.Gelu
                nc.scalar.activation(
                    gl[:, :], f1[:, :], gelu_f, bias=ct["b1"][:, fc : fc + 1]
                )
                nc.tensor.matmul(
                    f2[:, :],
                    ct["w2t"][:, 128 * fc : 128 * (fc + 1)],
                    gl[:, :],
                    start=(fc == 0),
                    stop=(fc == 3),
                )
            resid(h3[:, sl], f2[:, :], ct["b2"][:, :], h2[:, sl])

        # ---- op-mix 2 + output -------------------------------------------
        h4 = hpool.tile([D, TOK], F32, tag="h4")
        opmix(h3, "wop2", "ob2", 3, h4, "t3")
        nc.sync.dma_start(p["outT"][:, :], h4[:, :])

    nc.compile()
    return nc


# ---------------------------------------------------------------------------
# entry point
# ---------------------------------------------------------------------------
def kernel(**inputs):
    from concourse.bass_utils import run_bass_kernel_spmd

    if "nc" not in _CACHE:
        _CACHE["nc"] = build_kernel()
    nc = _CACHE["nc"]

    w = _prep_weights(inputs)
    in_maps = _per_core_inputs(inputs, w)
    res = run_bass_kernel_spmd(nc, in_maps, list(range(NCORES))).results

    out = np.empty((B, N, D), np.float32)
    for c in range(NCORES):
        b, half = c // 2, c % 2
        out[b, half * TOK : (half + 1) * TOK, :] = np.asarray(res[c]["outT"]).T
    return out
